# revision 41
# baseline (speedup 1.0000x reference)
"""CapsuleLayer dynamic-routing kernel for 8 Trainium2 NeuronCores.

Problem: inputs [64, 4096, 8] f32, W [32, 4096, 16, 8] f32.
  hat[b,c,n,j] = sum_i W[c,n,j,i] * x[b,n,i]
  3 routing iterations: c = softmax_C(b); out = squash(sum_n c*hat);
  b += <out, hat>_j.

Strategy: shard the n (input-capsule) axis across the 8 cores
(N_loc = 512/core).  Everything (W shard 8.4MB + x 1MB + workspace)
stays SBUF-resident; hat is never materialized.  Per routing iteration:
  - logits beta = <outsum, hat> via a block-diagonal zero-padded matmul
    (K = 8 capsules x 16 j = 128) producing A[b,c,n,i] = sum_j out*W,
    then a DVE multiply by x and an i-tree-reduction.
  - softmax over capsules is local per (b,n); exp on ScalarE; the 1/Z
    normalization is folded into x (x-tilde), so couplings stay
    unnormalized.
  - s partial = sum_n coupling*hat via per-capsule matmuls with
    K = n (128-row tiles), accumulating (i, n-tile) in PSUM.
  - one tiny [64,32,16] AllReduce per iteration; squash computed
    identically on every core.

Since the logit update is linear in out, b_t = <sum_{t'<t} out_t', hat>,
so logits are recomputed from the running sum each iteration (no [B,C,N]
logit state).
"""

import numpy as np

B, N, I = 64, 4096, 8
C, D = 32, 16
ROUTINGS = 3
EPS = 1e-7
NCORES = 8
NL = N // NCORES          # 512 n per core
NT = NL // 128            # 4 partition tiles of n
CHUNKS = NL * I // 512    # 8 chunks of 512 along flat (n,i)


# ---------------------------------------------------------------------------
# Host-side layout prep (pure numpy, per core)
# ---------------------------------------------------------------------------

def host_prep(x, W, k):
    """Per-core input layouts for core k (n slice [k*NL, (k+1)*NL))."""
    n0 = k * NL
    Wk = np.ascontiguousarray(W[:, n0:n0 + NL])          # [C, NL, D, I]
    xk = np.ascontiguousarray(x[:, n0:n0 + NL])          # [B, NL, I]

    # W2 [128=(cp*16+j), (cg, n*8+i)]  = W[cg*8+cp, n, j, i]   (bf16)
    w2 = Wk.reshape(4, 8, NL, D, I).transpose(1, 3, 0, 2, 4).reshape(128, 4 * NL * I)
    # W3 [128=nn, (nt, i, c, j)] = W[c, nt*128+nn, j, i]        (bf16)
    w3 = Wk.reshape(C, NT, 128, D, I).transpose(2, 1, 4, 0, 3).reshape(128, NT * I * C * D)
    # xt3 [128=nn, (i, nt, b)] = x[b, nt*128+nn, i]             (bf16)
    xt3 = xk.reshape(B, NT, 128, I).transpose(2, 3, 1, 0).reshape(128, I * NT * B)
    # xr2 [128=(h*64+b), (n*8+i)] = x[b, n, i]                  (bf16)
    xr2 = np.tile(xk.reshape(B, NL * I), (2, 1))

    import ml_dtypes
    bf = ml_dtypes.bfloat16
    return {
        "w2": w2.astype(bf),
        "w3": w3.astype(bf),
        "xt3": xt3.astype(bf),
        "xr2": xr2.astype(bf),
        "eyef": np.eye(128, dtype=np.float32),
        "bdmask": _bd_mask().astype(bf),
    }


def _bd_mask():
    # mask[r, col] = 1 where ((r%32)//16) == col//64 — selects which b-half
    # of a block-diagonal lhsT tile each 16-row (one capsule's j-block) feeds.
    r = np.arange(128)[:, None]
    col = np.arange(128)[None, :]
    return (((r % 32) // 16) == (col // 64)).astype(np.float32)


# ---------------------------------------------------------------------------
# Numpy emulation of the exact device dataflow (for layout validation)
# ---------------------------------------------------------------------------

def _squash_np(s):
    # s [B, C*D] -> squash over j
    s3 = s.reshape(B, C, D)
    s2 = (s3 * s3).sum(-1)                     # [B, C]
    q = np.sqrt(s2 + EPS)
    fac = s2 / ((1.0 + s2) * q)                # [B, C]
    return (s3 * fac[:, :, None]).reshape(B, C * D)


def emulate(x, W):
    """Mirror the device program slice-for-slice in numpy (f32)."""
    per_core = [
        {k: v.astype(np.float32) for k, v in host_prep(x, W, c).items()}
        for c in range(NCORES)
    ]
    sST = [np.zeros((128, 4, 64), np.float32) for _ in range(NCORES)]
    sET = [np.zeros((128, NT, C, 64), np.float32) for _ in range(NCORES)]
    sOsum = [np.zeros((B, C * D), np.float32) for _ in range(NCORES)]
    out_t = None

    for t in range(ROUTINGS):
        for k in range(NCORES):
            io = per_core[k]
            w3 = io["w3"].reshape(128, NT, I, C, D)
            xt3 = io["xt3"].reshape(128, I, NT, B)
            if t > 0:
                # (a) outsumT [128=(cp,j), (m, b)]; m = c-group of 8
                osumT = np.zeros((128, 4, 64), np.float32)
                for m in range(4):
                    blk = sOsum[k][:, 128 * m:128 * (m + 1)]    # [64, 128]
                    osumT[:, m, :] = blk.T
                # BD tiles [g][p]: [128, 128]
                BD = np.zeros((4, 4, 128, 128), np.float32)
                for g in range(4):
                    for p in range(4):
                        BD[g, p, 32 * p:32 * p + 16, 0:64] = osumT[32 * p:32 * p + 16, g, :]
                        BD[g, p, 32 * p + 16:32 * p + 32, 64:128] = osumT[32 * p + 16:32 * p + 32, g, :]
                # (b) A-matmuls + beta + e + ET
                w2 = io["w2"].reshape(128, 4, CHUNKS, 512)
                xr2 = io["xr2"].reshape(128, CHUNKS, 512)
                for g in range(4):
                    for p in range(4):
                        tmp = np.zeros((128, CHUNKS, 512), np.float32)
                        for ch in range(CHUNKS):
                            pA = BD[g, p].T @ w2[:, g, ch, :]   # [128=(cp2,b), 512]
                            tmp[:, ch, :] = pA * xr2[:, ch, :]
                        t8 = tmp.reshape(128, NL, I)
                        beta = t8.sum(-1)                        # [128, 512]
                        erow = np.exp(beta)
                        for nt in range(4):
                            blk = erow[:, 128 * nt:128 * (nt + 1)].T  # [128n, 128(cp2,b)]
                            c0 = g * 8 + 2 * p
                            sET[k][:, nt, c0:c0 + 2, :] = blk.reshape(128, 2, 64)
                # (c) Z, Zr, x-tilde
                Z = sET[k].transpose(0, 1, 3, 2).sum(-1)         # [128, nt, b]
                Zr = 1.0 / Z
                xtl = xt3 * Zr[:, None, :, :]                    # [128, i, nt, b]
            # (d) s-matmuls
            for cb in range(4):
                acc = np.zeros((128, 64), np.float32)
                if t == 0:
                    for i in range(I):
                        for nt in range(NT):
                            lhs = w3[:, nt, i, cb * 8:(cb + 1) * 8, :].reshape(128, 128)
                            acc += lhs.T @ xt3[:, i, nt, :]
                else:
                    for i in range(I):
                        for nt in range(NT):
                            et = sET[k][:, nt, cb * 8:(cb + 1) * 8, :]       # [128, 8, 64]
                            Rg = et * xtl[:, i, nt, None, :]                 # [128, 8, 64]
                            for c8 in range(8):
                                lhs = w3[:, nt, i, cb * 8 + c8, :]           # [128, 16]
                                acc[c8 * 16:(c8 + 1) * 16, :] += lhs.T @ Rg[:, c8, :]
                sST[k][:, cb, :] = acc * (1.0 / C if t == 0 else 1.0)
            # (e) transpose sST -> s_pre [64, (c,j)]
        # all-reduce
        s_pre = np.zeros((NCORES, B, C * D), np.float32)
        for k in range(NCORES):
            for cb in range(4):
                s_pre[k][:, cb * 128:(cb + 1) * 128] = sST[k][:, cb, :].T
        s_red = s_pre.sum(0)
        out_t = _squash_np(s_red)
        for k in range(NCORES):
            if t == 0:
                sOsum[k] = out_t.copy()
            elif t == 1:
                sOsum[k] = sOsum[k] + out_t
    return out_t.reshape(B, C, D)


# ---------------------------------------------------------------------------
# Bass device program
# ---------------------------------------------------------------------------

_CACHE = {}


DVE_DIRECT_MOD = 4
WP_BUFS = 2
PSA_BUFS = 3
TREE_L2_ENG = lambda nc: nc.vector


def _build_nc(sim=False, ablate=()):
    import concourse.bass as bass
    import concourse.bacc as bacc
    import concourse.mybir as mybir
    import concourse.tile as tile

    dt = mybir.dt
    f32, bf16 = dt.float32, dt.bfloat16
    ALU = mybir.AluOpType
    AF = mybir.ActivationFunctionType
    AX = mybir.AxisListType

    nc = bacc.Bacc("TRN2", target_bir_lowering=False, debug=False,
                   num_devices=NCORES)

    w2_d = nc.dram_tensor("w2", [128, 4 * NL * I], bf16, kind="ExternalInput").ap()
    w3_d = nc.dram_tensor("w3", [128, NT * I * C * D], bf16, kind="ExternalInput").ap()
    xt3_d = nc.dram_tensor("xt3", [128, I * NT * B], bf16, kind="ExternalInput").ap()
    xr2_d = nc.dram_tensor("xr2", [128, NL * I], bf16, kind="ExternalInput").ap()
    eyef_d = nc.dram_tensor("eyef", [128, 128], f32, kind="ExternalInput").ap()
    bdm_d = nc.dram_tensor("bdmask", [128, 128], bf16, kind="ExternalInput").ap()
    out_d = nc.dram_tensor("out", [B, C * D], f32, kind="ExternalOutput").ap()

    with tile.TileContext(nc) as tc:
        with (
            tc.tile_pool(name="const", bufs=1) as cp,
            tc.tile_pool(name="work", bufs=WP_BUFS) as wp,
            tc.tile_pool(name="dram", bufs=2, space="DRAM") as dp,
        ):
            sW2 = cp.tile([128, 4, CHUNKS, 512], bf16)
            sW3 = cp.tile([128, NT, I, C, D], bf16)
            sXT3 = cp.tile([128, I, NT, B], bf16)
            sXR2 = cp.tile([128, CHUNKS, 512], bf16)
            sEyeF = cp.tile([128, 128], f32)
            sBdm = cp.tile([128, 128], bf16)
            nc.sync.dma_start(sBdm[:], bdm_d[:])
            # spread the big input DMAs over distinct engine queues; W3/xT3
            # first (needed by the t=0 matmuls)
            if "nodma" in ablate:
                w3v = sW3[:].rearrange("p a b c d -> p (a b c d)")
                xt3v = sXT3[:].rearrange("p a b c -> p (a b c)")
                w2v = sW2[:].rearrange("p a b c -> p (a b c)")
                xr2v = sXR2[:].rearrange("p a b -> p (a b)")
                nc.sync.dma_start(w3v, w3_d[:].broadcast_to(w3v.shape))
                nc.scalar.dma_start(xt3v, xt3_d[:].broadcast_to(xt3v.shape))
                nc.gpsimd.dma_start(w2v, w2_d[:].broadcast_to(w2v.shape))
                nc.gpsimd.dma_start(xr2v, xr2_d[:].broadcast_to(xr2v.shape))
            else:
                nc.sync.dma_start(sW3[:].rearrange("p a b c d -> p (a b c d)"), w3_d[:])
                nc.scalar.dma_start(sXT3[:].rearrange("p a b c -> p (a b c)"), xt3_d[:])
                nc.gpsimd.dma_start(sW2[:].rearrange("p a b c -> p (a b c)"), w2_d[:])
                nc.gpsimd.dma_start(sXR2[:].rearrange("p a b -> p (a b)"), xr2_d[:])
            nc.scalar.dma_start(sEyeF[:], eyef_d[:])

            sET = cp.tile([128, NT, C, B], bf16)
            sXt = cp.tile([128, I, NT, B], bf16)
            sZ = cp.tile([128, NT, B], f32)
            sZr = cp.tile([128, NT, B], f32)
            sST = cp.tile([128, 4, B], f32)
            sSpre = cp.tile([B, C * D], f32)
            sS = cp.tile([B, C * D], f32)
            sOut = cp.tile([B, C * D], f32)
            sOsum = cp.tile([B, C * D], f32)
            sOsumT = cp.tile([128, 4, B], bf16)
            sBD = [[cp.tile([128, 128], bf16, name=f"bd{g}{p}", tag=f"bd{g}{p}")
                    for p in range(4)]
                   for g in range(4)]
            sRg = cp.tile([128, I, NT, 8, B], bf16)
            # squash temps
            sq = cp.tile([B, C * D], f32)
            s2 = cp.tile([B, C], f32)
            s2e = cp.tile([B, C], f32)
            q = cp.tile([B, C], f32)
            rq = cp.tile([B, C], f32)
            q2 = cp.tile([B, C], f32)
            qs = cp.tile([B, C], f32)
            opp = cp.tile([B, C], f32)
            den = cp.tile([B, C], f32)
            rden = cp.tile([B, C], f32)
            fac = cp.tile([B, C], f32)

            for g in range(4):
                for p in range(4):
                    nc.vector.memset(sBD[g][p][:], 0.0)

            def squash(src, dst):
                nc.vector.tensor_mul(sq[:], src[:], src[:])
                nc.vector.tensor_reduce(
                    s2[:], sq[:].rearrange("b (c j) -> b c j", j=D),
                    axis=AX.X, op=ALU.add)
                nc.vector.tensor_scalar_add(s2e[:], s2[:], EPS)
                nc.scalar.sqrt(q[:], s2e[:])
                nc.vector.reciprocal(rq[:], q[:])
                nc.vector.tensor_mul(q2[:], s2e[:], rq[:])
                nc.vector.tensor_add(qs[:], q[:], q2[:])          # 2*sqrt refined
                nc.vector.tensor_scalar_add(opp[:], s2[:], 1.0)
                nc.vector.tensor_mul(den[:], opp[:], qs[:])       # 2*(1+s2)*q
                nc.vector.reciprocal(rden[:], den[:])
                nc.vector.tensor_mul(fac[:], s2[:], rden[:])
                nc.vector.tensor_scalar_mul(fac[:], fac[:], 2.0)
                fb = fac[:].rearrange("b (c o) -> b c o", o=1).broadcast_to([B, C, D])
                nc.vector.tensor_mul(
                    dst[:].rearrange("b (c j) -> b c j", j=D),
                    src[:].rearrange("b (c j) -> b c j", j=D), fb)

            n_rout = 1 if "r1" in ablate else (2 if "r2" in ablate else ROUTINGS)
            for t in range(n_rout):
                if t > 0:
                    # (a) transpose outsum, build block-diagonal lhsT tiles
                    with tc.tile_pool(name="psO", bufs=2, space="PSUM") as psO:
                        for m in range(4):
                            pT = psO.tile([128, B], f32, tag="ot")
                            nc.tensor.transpose(
                                pT[:], sOsum[:, 128 * m:128 * (m + 1)],
                                sEyeF[0:B, 0:B])
                            nc.scalar.copy(sOsumT[:, m, :], pT[:])
                    for g in range(4):
                        for p in range(4):
                            ob = sOsumT[32 * p:32 * p + 32, g, :] \
                                .rearrange("p (o b) -> p o b", o=1) \
                                .broadcast_to([32, 2, B])
                            nc.vector.tensor_mul(
                                sBD[g][p][32 * p:32 * p + 32, :]
                                    .rearrange("p (h b) -> p h b", h=2),
                                ob,
                                sBdm[32 * p:32 * p + 32, :]
                                    .rearrange("p (h b) -> p h b", h=2))
                    # (b) A matmuls -> beta -> exp -> ET
                    with (
                        tc.tile_pool(name="psA", bufs=PSA_BUFS, space="PSUM") as psA,
                        tc.tile_pool(name="psE", bufs=2, space="PSUM") as psE,
                    ):
                        for g in range(4):
                            for p in range(4):
                                tmp = wp.tile([128, CHUNKS, 512], bf16, tag="tmp")
                                tmpf = wp.tile([128, CHUNKS, 512], bf16, tag="tmpf")
                                for c2 in range(CHUNKS // 2):
                                    pA = psA.tile([128, 1024], f32, tag="pA")
                                    for h in range(2):
                                        ch = 2 * c2 + h
                                        nc.tensor.matmul(
                                            pA[:, 512 * h:512 * (h + 1)],
                                            sBD[g][p][:], sW2[:, g, ch, :],
                                            start=True, stop=True)
                                    unit = (g * 4 + p) * 4 + c2
                                    tv = tmp[:, 2 * c2:2 * c2 + 2, :] \
                                        .rearrange("p a b -> p (a b)")
                                    xv = sXR2[:, 2 * c2:2 * c2 + 2, :] \
                                        .rearrange("p a b -> p (a b)")
                                    if "bmul" in ablate and c2 > 0:
                                        pass
                                    elif unit % DVE_DIRECT_MOD == 0:
                                        # direct 1x multiply from PSUM on DVE
                                        nc.vector.tensor_mul(tv, pA[:], xv)
                                    else:
                                        # ACT drain to bf16, then 2x DVE mul
                                        tfv = tmpf[:, 2 * c2:2 * c2 + 2, :] \
                                            .rearrange("p a b -> p (a b)")
                                        nc.scalar.copy(tfv, pA[:])
                                        nc.vector.tensor_mul(tv, tfv, xv)
                                t8 = tmp[:].rearrange("p a b -> p (a b)") \
                                           .rearrange("p (n i) -> p n i", i=I)
                                tr1 = wp.tile([128, NL, 4], bf16, tag="tr1")
                                tr2 = wp.tile([128, NL, 2], bf16, tag="tr2")
                                beta = wp.tile([128, NL, 1], f32, tag="beta")
                                if "tree" not in ablate:
                                    nc.vector.tensor_add(tr1[:], t8[:, :, 0:4], t8[:, :, 4:8])
                                    TREE_L2_ENG(nc).tensor_add(tr2[:], tr1[:, :, 0:2], tr1[:, :, 2:4])
                                    nc.gpsimd.tensor_add(beta[:], tr2[:, :, 0:1], tr2[:, :, 1:2])
                                else:
                                    nc.gpsimd.tensor_add(
                                        beta[:], t8[:, :, 0:1], t8[:, :, 1:2])
                                c0 = g * 8 + 2 * p
                                bv = beta[:].rearrange("p a b -> p (a b)")
                                for nt in range(4):
                                    pT2 = psE.tile([128, 128], f32, tag="eT")
                                    nc.tensor.transpose(
                                        pT2[:], bv[:, 128 * nt:128 * (nt + 1)],
                                        sEyeF[:])
                                    # exp applied on the transposed logits,
                                    # PSUM -> sET directly
                                    nc.scalar.activation(
                                        sET[:, nt, c0:c0 + 2, :]
                                           .rearrange("p a b -> p (a b)"),
                                        pT2[:], AF.Exp)
                    # (c) Z = sum_c e ; x-tilde = xt3 / Z
                    nc.vector.tensor_reduce(
                        sZ[:], sET[:].rearrange("p nt c b -> p nt b c"),
                        axis=AX.X, op=ALU.add)
                    nc.vector.reciprocal(sZr[:], sZ[:])
                    zb = sZr[:].rearrange("p (o nt) b -> p o nt b", o=1) \
                               .broadcast_to([128, I, NT, B])
                    nc.vector.tensor_mul(sXt[:], sXT3[:], zb)
                # (d) s matmuls
                for cb in range(4):
                    if t > 0 and "rg" not in ablate:
                        for i in range(I):
                            xb = sXt[:, i, :, :] \
                                .rearrange("p nt (o b) -> p nt o b", o=1) \
                                .broadcast_to([128, NT, 8, B])
                            nc.vector.tensor_mul(
                                sRg[:, i, :, :, :],
                                sET[:, :, cb * 8:(cb + 1) * 8, :], xb)
                    elif t > 0:
                        nc.vector.tensor_copy(
                            sRg[:].rearrange("p a b c d -> p (a b c d)"),
                            sW3[:].rearrange("p a b c d -> p (a b c d)"))
                    with tc.tile_pool(name=f"psS{t}{cb}", bufs=1, space="PSUM") as psS:
                        if t == 0:
                            pacc = psS.tile([128, B], f32, tag="s8")
                            step = 0
                            for i in range(I):
                                for nt in range(NT):
                                    lhs = sW3[:, nt, i, cb * 8:(cb + 1) * 8, :] \
                                        .rearrange("p a b -> p (a b)")
                                    nc.tensor.matmul(
                                        pacc[:], lhs, sXT3[:, i, nt, :],
                                        start=(step == 0), stop=(step == 31))
                                    step += 1
                            nc.scalar.mul(sST[:, cb, :], pacc[:], 1.0 / C)
                        else:
                            paccs = [psS.tile([B, D], f32, name=f"pacc{c8}",
                                              tag=f"s{c8}")
                                     for c8 in range(8)]
                            step = 0
                            nsteps = 2 if "smm" in ablate else 32
                            for i in range(I if "smm" not in ablate else 1):
                                for nt in range(NT if "smm" not in ablate else 2):
                                    for c8 in range(8):
                                        nc.tensor.matmul(
                                            paccs[c8][:],
                                            sRg[:, i, nt, c8, :],
                                            sW3[:, nt, i, cb * 8 + c8, :],
                                            start=(step == 0),
                                            stop=(step == nsteps - 1))
                                    step += 1
                            for c8 in range(8):
                                c = cb * 8 + c8
                                nc.scalar.copy(
                                    sSpre[:, c * D:(c + 1) * D], paccs[c8][:])
                # (e) transpose sST -> sSpre [64, (c,j)]  (t=0 path only)
                if t == 0:
                    with tc.tile_pool(name=f"psT{t}", bufs=2, space="PSUM") as psT:
                        for cb in range(4):
                            pT3 = psT.tile([B, 128], f32, tag="sT")
                            nc.tensor.transpose(pT3[:], sST[:, cb, :], sEyeF[:])
                            nc.scalar.copy(sSpre[:, cb * 128:(cb + 1) * 128], pT3[:])
                # all-reduce s across cores
                if sim:
                    nc.vector.tensor_copy(sS[:], sSpre[:])
                else:
                    di = dp.tile([B, C * D], f32, tag="ar_in")
                    do = dp.tile([B, C * D], f32, tag="ar_out")
                    nc.sync.dma_start(di[:], sSpre[:])
                    nc.gpsimd.collective_compute(
                        "AllReduce", mybir.AluOpType.add,
                        replica_groups=[list(range(NCORES))],
                        ins=[di[:].opt()], outs=[do[:].opt()])
                    nc.sync.dma_start(sS[:], do[:])
                squash(sS, sOut)
                if t == n_rout - 1:
                    nc.sync.dma_start(out_d[:], sOut[:])
                elif t == 0:
                    nc.vector.tensor_copy(sOsum[:], sOut[:])
                else:
                    nc.vector.tensor_add(sOsum[:], sOsum[:], sOut[:])
    nc.compile()
    return nc


def get_nc(sim=False, ablate=()):
    key = ("nc_sim" if sim else "nc") + "_".join(ablate)
    if key not in _CACHE:
        _CACHE[key] = _build_nc(sim=sim, ablate=ablate)
    return _CACHE[key]


def kernel(inputs, W):
    inputs = np.asarray(inputs, dtype=np.float32)
    W = np.asarray(W, dtype=np.float32)
    nc = get_nc()
    in_maps = [host_prep(inputs, W, k) for k in range(NCORES)]
    from concourse import bass_utils
    res = bass_utils.run_bass_kernel_spmd(
        nc, in_maps, core_ids=list(range(NCORES)))
    return res.results[0]["out"].reshape(B, C, D).astype(np.float32)
